# revision 2
# baseline (speedup 1.0000x reference)
"""Trainium2 Bass kernel for nn_AutoRegressive_45234595562178 — v4.

Width-schedule / sorted-column / snapshot-gather skeleton, L=2 lanes.
Key structure (per lane, per step):

  - ONE gates matmul: lhsT = [x-w(13); bias; 0-pad; h-w(32)] at PE row-group
    64, rhs = state tile rows 64:128 = [x(13); 1; 0-pad(18); h'(32)].
    x and the ones row are DMA'd per chunk straight into the state tile
    (host ships xt with a 14th all-ones row); h' is written there by the
    h-STT of the previous step.  This removes the separate x-part matmul
    and its LDWEIGHTS from the PE entirely.
  - tanh-trick cell: ONE gate activation (tanh of all 4 gate blocks), g~
    staged to PSUM cb rows 32:64 by the selG matmul, stacked STT for
    u=(f~+1)*C2 and v=(i~+1)*g~, compA matmul for C2'=0.5u+v, tanh(c),
    h' = (o~+1)*tct.
  - snapshots: gpsimd copies of the dying column window, uv (rows 0:64 of
    the per-step uv tile) and h' (rows 96:128 of the state tile) into one
    [128, S_pad] snap buffer (rows 64:96 unused); gather via PE transpose
    + permutation matmul into a [128, BPC] PSUM accumulator.
  - decode: per-lane state tile [77, HL] = [h'(32); 1; 0-pad(31); elem(13)]
    (elem constant per sequence), ONE K=77 gates matmul per lane; pred =
    Wd33 @ [h'; 1] with folded bias, copied to SBUF bf16 on the DVE and
    DMA'd out in groups of 4 steps (host upcasts to fp32).

State: h' = 2h; C2 = 2c in per-lane single-buffered PSUM cb tiles.
"""

import numpy as np
import ml_dtypes

D_IN, D_H, B, T, MAX_OUT, NCORES = 13, 32, 4096, 512, 256, 8
BPC = B // NCORES
G4 = 4 * D_H
SNAP_PAD = 4
X_CHUNK = 32
DEC_GROUP = 4

BF16 = ml_dtypes.bfloat16

ROWS = dict(f=0, i=32, o=64, g=96)


def make_schedules(lengths, out_steps, L=2):
    HL = BPC // L

    def r4(n):
        return min(HL, -(-n // 4) * 4)

    steps = np.clip(np.asarray(lengths).astype(np.int64), 1, T)
    dec = np.clip(np.asarray(out_steps).astype(np.int64), 1, MAX_OUT)

    order = np.argsort(-steps, kind="stable")
    assign = np.stack(
        [np.concatenate([order[c::NCORES][l::L] for l in range(L)])
         for c in range(NCORES)])
    steps_pc = steps[assign]

    Tmax = int(steps.max())
    counts = np.bincount(steps, minlength=T + 2)
    surv = B - np.cumsum(counts)
    Wh = np.array([r4(-(-int(surv[t]) // (NCORES * L))) for t in range(Tmax)],
                  np.int64)
    tgrid = np.arange(Tmax)[:, None]
    for c in range(NCORES):
        for l in range(L):
            scol = steps_pc[c, l * HL:(l + 1) * HL]
            n_ct = (scol[None, :] > tgrid).sum(1)
            assert np.all(Wh >= n_ct), "width schedule violates actives"

    Whnext = np.append(Wh[1:], 0)
    lo = np.maximum(0, Whnext - SNAP_PAD)
    wwin = Wh - lo
    off = np.concatenate([[0], np.cumsum(L * wwin)])
    S = int(off[-1])
    S_pad = -(-S // 128) * 128

    slot = np.zeros((NCORES, BPC), np.int64)
    for c in range(NCORES):
        for l in range(L):
            scol = steps_pc[c, l * HL:(l + 1) * HL]
            tprime = scol - 1
            j = np.arange(HL)
            assert np.all(j >= lo[tprime]) and np.all(j < Wh[tprime])
            slot[c, l * HL:(l + 1) * HL] = (
                off[tprime] + l * wwin[tprime] + (j - lo[tprime]))

    dec_pc = dec[assign]
    dorder = np.zeros((NCORES, BPC), np.int64)
    for c in range(NCORES):
        didx = np.argsort(-dec_pc[c], kind="stable")
        dorder[c] = np.concatenate([didx[l::L] for l in range(L)])
    dec_at = np.take_along_axis(dec_pc, dorder, axis=1)
    Ms = np.zeros((L, MAX_OUT), np.int64)
    Ms[:, 0] = HL
    for s in range(1, MAX_OUT):
        for l in range(L):
            Ms[l, s] = r4(int((dec_at[:, l * HL:(l + 1) * HL] > s).sum(1).max()))
    for l in range(1, L):
        assert np.all(Ms[l - 1] >= Ms[l])
    Smax = int(np.nonzero(Ms[0])[0].max()) + 1

    pmat = np.zeros((NCORES, S_pad, BPC), np.float32)
    for c in range(NCORES):
        pmat[c, slot[c][dorder[c]], np.arange(BPC)] = 1.0

    return dict(
        steps=steps, dec=dec, assign=assign, steps_pc=steps_pc, Tmax=Tmax,
        Wh=Wh, lo=lo, wwin=wwin, off=off, S=S, S_pad=S_pad, slot=slot,
        dorder=dorder, dec_pc=dec_pc, Ms=Ms, Smax=Smax, pmat=pmat, L=L, HL=HL,
    )


def prep_weights(W_ih, W_hh, b_ih, b_hh, Wd, bd):
    """Scale-folded weights, gate order [f, i, o, g]."""
    rs = np.ones(G4, np.float32) * 0.5
    rs[64:96] = 1.0  # g rows keep full scale in natural [i,f,g,o] order
    Wx_n = (rs[:, None] * np.asarray(W_ih, np.float32)).T        # [13, 128]
    Wh_n = (rs[:, None] * 0.5 * np.asarray(W_hh, np.float32)).T  # [32, 128]
    bias_n = rs * (np.asarray(b_ih, np.float32) + np.asarray(b_hh, np.float32))

    nat = dict(i=0, f=32, g=64, o=96)
    p = np.zeros(G4, np.int64)
    for gname in ("i", "f", "g", "o"):
        p[ROWS[gname]:ROWS[gname] + 32] = np.arange(nat[gname],
                                                    nat[gname] + 32)
    Wx = Wx_n[:, p]
    Wh_ = Wh_n[:, p]
    bias_p = bias_n[p]

    # warmup merged weight for state rows 64:128 = [x(13); 1; 0(18); h'(32)]
    Wwarm = np.zeros((64, G4), np.float32)
    Wwarm[0:13] = Wx
    Wwarm[13] = bias_p
    Wwarm[32:64] = Wh_

    # decode merged weight for state rows [h'(32); 1; 0(31); elem(13)]
    Wfull = np.zeros((77, G4), np.float32)
    Wfull[0:32] = Wh_
    Wfull[32] = bias_p
    Wfull[64:77] = Wx

    # prediction weight with bias row: pred = [h'; 1] @ Wd33
    Wd33 = np.zeros((D_H + 1, D_IN), np.float32)
    Wd33[0:32] = (0.5 * np.asarray(Wd, np.float32)).T
    Wd33[32] = np.asarray(bd, np.float32)

    compA = np.zeros((64, 32), np.float32)      # [u; v] -> 0.5 u + v
    compA[0:32] = 0.5 * np.eye(32)
    compA[32:64] = np.eye(32)
    selG = np.zeros((G4, 32), np.float32)
    selG[ROWS["g"]:ROWS["g"] + 32] = np.eye(32)
    return Wwarm, Wfull, Wd33, compA, selG


def _split_sync_waits(m):
    import bass_rust
    import concourse.mybir as mybir
    ctr = [0]
    for fn in m.functions:
        for bb in fn.blocks:
            out_list = []
            changed = False
            for inst in bb.instructions:
                si = inst.sync_info
                waits = list(si.on_wait) if si is not None else []
                if len(waits) > 1:
                    changed = True
                    for w in waits[:-1]:
                        ctr[0] += 1
                        nop = mybir.InstNoOp(
                            name=f"wsplit-{ctr[0]}", ins=[], outs=[])
                        nop.engine = inst.engine
                        nop.sync_info = bass_rust.SyncInfo(
                            on_wait=[w], on_update=[])
                        out_list.append(nop)
                    si.on_wait = waits[-1:]
                out_list.append(inst)
            if changed:
                bb.instructions = out_list


def _build_program(sch, reps=1):
    import concourse.bass as bass
    import concourse.mybir as mybir
    from concourse.tile import TileContext

    fp32 = mybir.dt.float32
    bf16 = mybir.dt.bfloat16
    ADD = mybir.AluOpType.add
    MULT = mybir.AluOpType.mult
    TANH = mybir.ActivationFunctionType.Tanh

    Tmax, Wh, lo, wwin, off = (sch["Tmax"], sch["Wh"], sch["lo"], sch["wwin"],
                               sch["off"])
    S_pad, Ms, Smax = sch["S_pad"], sch["Ms"], sch["Smax"]
    L, HL = sch["L"], sch["HL"]
    KCH = S_pad // 128
    OB = ROWS["o"]
    CH = X_CHUNK

    nc = bass.Bass("TRN2", target_bir_lowering=False)
    xt = nc.dram_tensor("xt", [T, D_IN + 1, BPC], bf16, kind="ExternalInput")
    ww_d = nc.dram_tensor("wwarm", [64, G4], bf16, kind="ExternalInput")
    wf_d = nc.dram_tensor("wfull", [77, G4], bf16, kind="ExternalInput")
    ca_d = nc.dram_tensor("compA", [64, 32], bf16, kind="ExternalInput")
    sg_d = nc.dram_tensor("selG", [G4, 32], bf16, kind="ExternalInput")
    wd_d = nc.dram_tensor("wd33", [D_H + 1, D_IN], bf16, kind="ExternalInput")
    pm_d = nc.dram_tensor("pmat", [S_pad, BPC], bf16, kind="ExternalInput")
    id_d = nc.dram_tensor("ident", [128, 128], bf16, kind="ExternalInput")
    out_d = nc.dram_tensor("out", [MAX_OUT, D_IN, BPC], bf16,
                           kind="ExternalOutput")

    with TileContext(nc) as tc:
        with (
            tc.tile_pool(name="consts", bufs=1) as cpool,
            tc.tile_pool(name="state", bufs=1) as spool,
            tc.tile_pool(name="gates", bufs=5) as gpool,
            tc.tile_pool(name="vtmp", bufs=5) as vpool,
            tc.tile_pool(name="uvt", bufs=4) as upool,
            tc.tile_pool(name="outs", bufs=2) as opool,
            tc.tile_pool(name="pmchunk", bufs=2) as pmpool,
            tc.tile_pool(name="snapT", bufs=3) as stpool,
            tc.tile_pool(name="pgates", bufs=1, space="PSUM") as pgpool,
            tc.tile_pool(name="ppred", bufs=1, space="PSUM") as prpool,
            tc.tile_pool(name="pcb", bufs=1, space="PSUM") as cbpool,
            tc.tile_pool(name="pacc", bufs=1, space="PSUM") as papool,
        ):
            def emit_body():
                # warm weights at partitions 64:128 to match the rhs base
                wwarm = cpool.tile([128, G4], bf16, name="wwarm")
                nc.sync.dma_start(wwarm[64:128, :], ww_d[:])
                wfull = cpool.tile([77, G4], bf16, name="wfull")
                nc.sync.dma_start(wfull[:], wf_d[:])
                selGt = cpool.tile([G4, 32], bf16, name="selG")
                nc.sync.dma_start(selGt[:], sg_d[:])
                compAt = cpool.tile([64, 32], bf16)
                nc.sync.dma_start(compAt[:], ca_d[:])
                wd_sb = cpool.tile([D_H + 1, D_IN], bf16)
                nc.sync.dma_start(wd_sb[:], wd_d[:])
                id_sb = cpool.tile([128, 128], bf16)
                nc.sync.dma_start(id_sb[:], id_d[:])

                # per-lane double-buffered state tiles:
                # rows 64:77 x, 77 ones, 78:96 zero, 96:128 h'
                sxh = [[spool.tile([128, CH, HL], bf16, name=f"sxh{l}_{b}")
                        for b in range(2)] for l in range(L)]
                for l in range(L):
                    for b in range(2):
                        nc.vector.memset(sxh[l][b][64:128, :, :], 0.0)
                cbs = []
                for l in range(L):
                    cb_l = cbpool.tile([64, HL], fp32, name=f"cb{l}")
                    nc.vector.memset(cb_l[:], 0.0)
                    cbs.append(cb_l)
                snap = spool.tile([128, S_pad], bf16)
                nc.vector.memset(snap[:], 0.0)

                def cell(jobs, hdst):
                    """jobs: (W, lhsT, rhs, lane); hdst[l] = (uv_ap, h_ap)."""
                    jobs = [j for j in jobs if j[0]]
                    pgs = {}
                    for W, kx, rx, l in jobs:
                        pgl = pgpool.tile([G4, HL], fp32, tag=f"pg{l}")
                        pgs[l] = pgl
                        nc.tensor.matmul(pgl[:, :W], kx, rx,
                                         start=True, stop=True)
                    tgs = {}
                    for W, kx, rx, l in jobs:
                        tg = gpool.tile([G4, HL], bf16, tag=f"tg{l}")
                        nc.scalar.activation(tg[:, :W], pgs[l][:, :W], TANH)
                        tgs[l] = tg
                    for W, kx, rx, l in jobs:
                        nc.tensor.matmul(cbs[l][32:64, :W],
                                         selGt[:], tgs[l][:, :W],
                                         start=True, stop=True)
                    uvs = {}
                    for W, kx, rx, l in jobs:
                        uv = hdst[l][0]
                        nc.vector.scalar_tensor_tensor(
                            uv, tgs[l][0:64, :W], 1.0,
                            cbs[l][:, :W], ADD, MULT)
                        uvs[l] = uv
                    for W, kx, rx, l in jobs:
                        nc.tensor.matmul(cbs[l][0:32, :W],
                                         compAt[:], uvs[l],
                                         start=True, stop=True)
                    tcts = {}
                    for W, kx, rx, l in jobs:
                        tct = vpool.tile([G4, HL], bf16, tag=f"tct{l}")
                        nc.scalar.activation(tct[OB:OB + 32, :W],
                                             cbs[l][0:32, :W],
                                             TANH, scale=0.5)
                        tcts[l] = tct
                    for W, kx, rx, l in jobs:
                        nc.vector.scalar_tensor_tensor(
                            hdst[l][1], tgs[l][OB:OB + 32, :W], 1.0,
                            tcts[l][OB:OB + 32, :W], ADD, MULT)

                # ---- warmup ----
                for t in range(Tmax):
                    b = (t // CH) % 2
                    tl = t % CH
                    if tl == 0:
                        ch = min(CH, Tmax - t)
                        for l in range(L):
                            nc.sync.dma_start(
                                sxh[l][b][64:78, :ch, :],
                                xt[t:t + ch, :, l * HL:(l + 1) * HL]
                                .rearrange("t d c -> d t c"))
                    W = int(Wh[t])
                    t1 = t + 1
                    b1, tl1 = (t1 // CH) % 2, t1 % CH
                    uvt = upool.tile([64, L, HL], bf16, tag="uvt")
                    hdst = [(uvt[:, l, :W], sxh[l][b1][96:128, tl1, :W])
                            for l in range(L)]
                    cell([(W, wwarm[64:128, :], sxh[l][b][64:128, tl, :W],
                           l) for l in range(L)], hdst)
                    lw, w, o = int(lo[t]), int(wwin[t]), int(off[t])
                    nc.gpsimd.tensor_copy(snap[0:64, o:o + L * w],
                                          uvt[:, :, lw:lw + w])
                    for l in range(L):
                        nc.gpsimd.tensor_copy(
                            snap[96:128, o + l * w:o + (l + 1) * w],
                            sxh[l][b1][96:128, tl1, lw:lw + w])

                # ---- gather snapshots into decode order ----
                acc = papool.tile([128, BPC], fp32, name="acc")
                for k in range(KCH):
                    pm_k = pmpool.tile([128, BPC], bf16, tag="pm")
                    nc.sync.dma_start(pm_k[:], pm_d[128 * k:128 * (k + 1), :])
                    pt = prpool.tile([128, 128], bf16, tag="pt")
                    nc.tensor.transpose(pt[:],
                                        snap[:, 128 * k:128 * (k + 1)],
                                        id_sb[:])
                    sT = stpool.tile([128, 128], bf16, tag="sT")
                    nc.scalar.copy(sT[:], pt[:])
                    nc.tensor.matmul(acc[:], sT[:], pm_k[:],
                                     start=(k == 0), stop=(k == KCH - 1))

                # decode state tiles: [h'(32); 1; 0(31); elem(13)]
                xhd = []
                for l in range(L):
                    xh = spool.tile([77, HL], bf16, name=f"xhd{l}")
                    nc.vector.memset(xh[32:64, :], 1.0)
                    xhd.append(xh)
                for l in range(L):
                    nc.scalar.copy(xhd[l][0:32, :],
                                   acc[96:128, l * HL:(l + 1) * HL])
                cpUV = stpool.tile([64, BPC], bf16, name="cpUV")
                nc.scalar.copy(cpUV[:], acc[0:64, :])
                for l in range(L):
                    nc.tensor.matmul(cbs[l][0:32, :], compAt[:],
                                     cpUV[:, l * HL:(l + 1) * HL],
                                     start=True, stop=True)

                # ---- element = h_sel @ Wd.T + bd (bias row folded) ----
                pe = prpool.tile([D_IN, BPC], fp32, tag="pp")
                for l in range(L):
                    nc.tensor.matmul(pe[:, l * HL:(l + 1) * HL], wd_sb[:],
                                     xhd[l][0:33, :], start=True, stop=True)
                po0 = opool.tile([D_IN, DEC_GROUP, BPC], bf16, tag="po")
                nc.vector.tensor_copy(po0[:, 0, :], pe[:])
                nc.sync.dma_start(out_d[0], po0[:, 0, :])
                for l in range(L):
                    nc.scalar.copy(xhd[l][64:77, :],
                                   pe[:, l * HL:(l + 1) * HL])

                # ---- autoregressive decode ----
                po = None
                s0 = span0 = 0
                for s in range(1, Smax):
                    Wl = [int(Ms[l, s]) for l in range(L)]
                    hdst = []
                    for l in range(L):
                        uvd = vpool.tile([64, HL], bf16, tag=f"uvd{l}")
                        hdst.append((uvd[:, :Wl[l]], xhd[l][0:32, :Wl[l]]))
                    cell([(Wl[l], wfull[:], xhd[l][:, :Wl[l]], l)
                          for l in range(L)], hdst)
                    pp = prpool.tile([D_IN, BPC], fp32, tag="pp")
                    for l in range(L):
                        if Wl[l]:
                            cs = l * HL
                            nc.tensor.matmul(pp[:, cs:cs + Wl[l]], wd_sb[:],
                                             xhd[l][0:33, :Wl[l]],
                                             start=True, stop=True)
                    span = HL + Wl[1] if Wl[1] else Wl[0]
                    if po is None:
                        po = opool.tile([D_IN, DEC_GROUP, BPC], bf16,
                                        tag="po")
                        s0, span0 = s, span
                    nc.vector.tensor_copy(po[:, (s - s0), :span],
                                          pp[:, :span])
                    if s - s0 == DEC_GROUP - 1 or s == Smax - 1:
                        g = s - s0 + 1
                        nc.sync.dma_start(
                            out_d[s0:s0 + g, :, 0:span0].rearrange(
                                "g d b -> d g b"),
                            po[:, 0:g, :span0])
                        po = None

            if reps == 1:
                emit_body()
            else:
                with tc.For_i(0, reps, 1):
                    emit_body()

    _split_sync_waits(nc.m)
    return nc


def _host_prep(x, lengths, out_steps, W_ih, W_hh, b_ih, b_hh, Wd, bd):
    x = np.asarray(x, np.float32)
    sch = make_schedules(lengths, out_steps, L=2)
    Wwarm, Wfull, Wd33, compA, selG = prep_weights(
        W_ih, W_hh, b_ih, b_hh, Wd, bd)
    ident = np.eye(128, dtype=np.float32).astype(BF16)
    in_maps = []
    for c in range(NCORES):
        xc = np.empty((T, D_IN + 1, BPC), BF16)
        xc[:, :D_IN, :] = x[sch["assign"][c]].transpose(1, 2, 0).astype(BF16)
        xc[:, D_IN, :] = BF16(1.0)
        im = {
            "xt": xc,
            "wwarm": Wwarm.astype(BF16),
            "wfull": Wfull.astype(BF16),
            "wd33": Wd33.astype(BF16),
            "compA": compA.astype(BF16),
            "selG": selG.astype(BF16),
            "pmat": np.ascontiguousarray(sch["pmat"][c]).astype(BF16),
            "ident": ident,
        }
        in_maps.append(im)
    return sch, in_maps


def _assemble(sch, results):
    out = np.zeros((B, MAX_OUT, D_IN), np.float32)
    ar = np.arange(MAX_OUT)
    for c in range(NCORES):
        dev = np.asarray(results[c]["out"], np.float32)
        ids = sch["assign"][c][sch["dorder"][c]]
        valid = ar[:, None] < sch["dec"][ids][None, :]
        dd = np.where(valid[:, None, :], dev, 0.0)
        out[ids] = dd.transpose(2, 0, 1)
    return out


def kernel(x, lengths, out_steps, max_out, W_ih, W_hh, b_ih, b_hh, Wd, bd):
    from concourse.bass_utils import run_bass_kernel_spmd

    assert int(max_out) == MAX_OUT
    sch, in_maps = _host_prep(x, lengths, out_steps, W_ih, W_hh, b_ih, b_hh,
                              Wd, bd)
    nc = _build_program(sch)
    res = run_bass_kernel_spmd(nc, in_maps, core_ids=list(range(NCORES)))
    return _assemble(sch, res.results)


def measure_hw_time(inputs, R=256, tries=5):
    import time
    from concourse.bass_utils import run_bass_kernel_spmd

    sch, in_maps = _host_prep(
        inputs["x"], inputs["lengths"], inputs["out_steps"], inputs["W_ih"],
        inputs["W_hh"], inputs["b_ih"], inputs["b_hh"], inputs["Wd"],
        inputs["bd"])
    cores = list(range(NCORES))
    ncs = {r: _build_program(sch, reps=r) for r in (1, R)}
    for r in (1, R):
        run_bass_kernel_spmd(ncs[r], in_maps, core_ids=cores)
    walls = {1: [], R: []}
    deltas = []
    for _ in range(tries):
        t0 = time.perf_counter()
        run_bass_kernel_spmd(ncs[1], in_maps, core_ids=cores)
        a = time.perf_counter() - t0
        t0 = time.perf_counter()
        run_bass_kernel_spmd(ncs[R], in_maps, core_ids=cores)
        b = time.perf_counter() - t0
        walls[1].append(a)
        walls[R].append(b)
        deltas.append((b - a) / (R - 1))
    pos = [d for d in deltas if d > 0]
    d = min(pos) if pos else (min(walls[R]) - min(walls[1])) / (R - 1)
    return d * 1e9, walls
